# revision 36
# baseline (speedup 1.0000x reference)
import sys

if "/opt/trn_rl_repo" not in sys.path:
    sys.path.insert(0, "/opt/trn_rl_repo")

import numpy as np
import ml_dtypes

BF16 = ml_dtypes.bfloat16
F8 = ml_dtypes.float8_e4m3
B, S, H = 2, 2048, 4096
NH, NKV, D = 32, 8, 128
T = B * S
KBLK = H // 128  # 32
SCALE = float(D) ** -0.5
NCORES = 8
XS = 32.0  # activation fp8 scale (2^5)
WS = 512.0  # weight fp8 scale (2^9)
INV_SCALE = 1.0 / (XS * WS)  # 2^-14

_NC = None


def build_nc():
    from concourse import bacc, tile, mybir

    dt = mybir.dt
    Act = mybir.ActivationFunctionType
    Alu = mybir.AluOpType
    PM = mybir.MatmulPerfMode

    nc = bacc.Bacc("TRN2", target_bir_lowering=False, debug=False, num_devices=NCORES)

    hsh_d = nc.dram_tensor("hsh", [128, KBLK, T], dt.float8e4, kind="ExternalInput")
    hsl_d = nc.dram_tensor("hsl", [128, KBLK, T], dt.float8e4, kind="ExternalInput")
    # cos/sin pre-scaled by 2^-14 on host (cancels the fp8 scale in q/k psums)
    cos_d = nc.dram_tensor("cosT", [128, T], dt.bfloat16, kind="ExternalInput")
    sin_d = nc.dram_tensor("sinT", [128, T], dt.bfloat16, kind="ExternalInput")
    wqh_d = nc.dram_tensor("wqh", [128, KBLK, 512], dt.float8e4, kind="ExternalInput")
    wql_d = nc.dram_tensor("wql", [128, KBLK, 512], dt.float8e4, kind="ExternalInput")
    wkh_d = nc.dram_tensor("wkh", [128, KBLK, 128], dt.float8e4, kind="ExternalInput")
    wkl_d = nc.dram_tensor("wkl", [128, KBLK, 128], dt.float8e4, kind="ExternalInput")
    wvh_d = nc.dram_tensor("wvh", [128, KBLK, 128], dt.float8e4, kind="ExternalInput")
    wvl_d = nc.dram_tensor("wvl", [128, KBLK, 128], dt.float8e4, kind="ExternalInput")
    woh_d = nc.dram_tensor("woh", [128, 4, H], dt.float8e4, kind="ExternalInput")
    wol_d = nc.dram_tensor("wol", [128, 4, H], dt.float8e4, kind="ExternalInput")
    tri_d = nc.dram_tensor("tri", [128, 128], dt.bfloat16, kind="ExternalInput")
    # y carries a 2^14 scale; host divides after the cross-core reduction
    y_d = nc.dram_tensor("y", [T, H], dt.bfloat16, kind="ExternalOutput")

    with tile.TileContext(nc) as tc:
        with tc.tile_pool(name="persist", bufs=1) as pp:
            # per-chunk tiles => precise (non-coarse) dependency tracking
            Qts = [
                [pp.tile([128, 512], dt.bfloat16, name=f"q{h}_{t}") for t in range(8)]
                for h in range(4)
            ]
            Kts = [pp.tile([128, 512], dt.bfloat16, name=f"k{t}") for t in range(8)]
            # V augmented with a 512-valued column at free idx 128: the
            # denominator then carries 512*sum(p) while numerator carries
            # 2^14*sum(p*v), so the normalized output lands pre-scaled by 2^5
            # (the fp8 scale wanted for the o_proj split) for free.
            Vts = [pp.tile([128, 132], dt.bfloat16, name=f"v{c}") for c in range(32)]
            OHts = [pp.tile([128, 4, 128], dt.float8e4, name=f"oh{i}") for i in range(32)]
            OLts = [pp.tile([128, 4, 128], dt.float8e4, name=f"ol{i}") for i in range(32)]
            tri_sb = pp.tile([128, 128], dt.bfloat16)
            for c in range(32):
                nc.vector.memset(Vts[c][:, 128:132], XS * WS / XS)  # 512.0

            defer7 = []
            # ---------------- Phase A: QKV projections + RoPE ----------------
            with (
                tc.tile_pool(name="aw", bufs=1) as aw,
                tc.tile_pool(name="slabp", bufs=2) as slabp,
                tc.tile_pool(name="ascr", bufs=3) as ascr,
                tc.tile_pool(name="psa", bufs=1, space="PSUM") as psa,
            ):
                wq_h = aw.tile([128, KBLK, 512], dt.float8e4)
                wq_l = aw.tile([128, KBLK, 512], dt.float8e4)
                wk_h = aw.tile([128, KBLK, 128], dt.float8e4)
                wk_l = aw.tile([128, KBLK, 128], dt.float8e4)
                wv_h = aw.tile([128, KBLK, 128], dt.float8e4)
                wv_l = aw.tile([128, KBLK, 128], dt.float8e4)

                def rope(hd, tb, asc, asw, cs, sn):
                    # dst[:64] = x[:64]*cos[:64] - x[64:]*sin[:64]
                    # dst[64:] = x[64:]*cos[64:] + x[:64]*sin[64:]
                    # asw holds x with the halves pre-swapped, so both muls
                    # and the sub/add are partition-aligned on DVE.
                    tmpc = ascr.tile([128, 512], dt.bfloat16, bufs=2)
                    tmps = ascr.tile([128, 512], dt.bfloat16, bufs=2)
                    nc.vector.tensor_mul(tmpc[:], asc[:], cs[:])
                    nc.vector.tensor_mul(tmps[:], asw[:], sn[:])
                    if hd is None:
                        d_lo = Kts[tb][0:64, :]
                        d_hi = Kts[tb][64:128, :]
                    else:
                        d_lo = Qts[hd][tb][0:64, :]
                        d_hi = Qts[hd][tb][64:128, :]
                    nc.vector.tensor_sub(d_lo, tmpc[0:64, :], tmps[0:64, :])
                    nc.vector.tensor_add(d_hi, tmpc[64:128, :], tmps[64:128, :])

                for tb in range(8):
                    c0 = tb * 512
                    kp = psa.tile([128, 512], dt.float32)
                    vtp = psa.tile([128, 512], dt.float32)
                    qps = [
                        psa.tile([128, 512], dt.float32, name=f"qp{_h}")
                        for _h in range(4)
                    ]
                    cos_sb = ascr.tile([128, 512], dt.bfloat16, name="cos_sb", bufs=2)
                    sin_sb = ascr.tile([128, 512], dt.bfloat16, name="sin_sb", bufs=2)
                    if tb > 0:
                        nc.sync.dma_start(cos_sb[:], cos_d[:, c0 : c0 + 512])
                        nc.sync.dma_start(sin_sb[:], sin_d[:, c0 : c0 + 512])
                    for half in range(2):
                        ko = half * 16
                        sl_h = slabp.tile([128, 16, 512], dt.float8e4, name="slh")
                        sl_l = slabp.tile([128, 16, 512], dt.float8e4, name="sll")
                        if tb == 0 and half == 0:
                            nc.sync.dma_start(sl_h[:, 0:2, :], hsh_d[:, 0:2, 0:512])
                            nc.sync.dma_start(wv_h[:, 0:4, :], wvh_d[:, 0:4, :])
                            nc.sync.dma_start(sl_h[:, 2:8, :], hsh_d[:, 2:8, 0:512])
                            nc.sync.dma_start(wv_h[:, 4:16, :], wvh_d[:, 4:16, :])
                            nc.sync.dma_start(sl_h[:, 8:16, :], hsh_d[:, 8:16, 0:512])
                            nc.sync.dma_start(wk_h[:, 0:16, :], wkh_d[:, 0:16, :])
                            nc.sync.dma_start(wq_h[:, 0:8, :], wqh_d[:, 0:8, :])
                            nc.sync.dma_start(wq_h[:, 8:16, :], wqh_d[:, 8:16, :])
                            nc.sync.dma_start(wv_l[:, 0:16, :], wvl_d[:, 0:16, :])
                            nc.sync.dma_start(wk_l[:, 0:16, :], wkl_d[:, 0:16, :])
                            nc.sync.dma_start(wq_l[:, 0:8, :], wql_d[:, 0:8, :])
                            nc.sync.dma_start(wq_l[:, 8:16, :], wql_d[:, 8:16, :])
                            nc.sync.dma_start(sl_l[:], hsl_d[:, 0:16, 0:512])
                        elif tb == 0 and half == 1:
                            nc.sync.dma_start(sl_h[:], hsh_d[:, 16:32, 0:512])
                            nc.sync.dma_start(wv_h[:, 16:32, :], wvh_d[:, 16:32, :])
                            nc.sync.dma_start(wk_h[:, 16:32, :], wkh_d[:, 16:32, :])
                            nc.sync.dma_start(wq_h[:, 16:24, :], wqh_d[:, 16:24, :])
                            nc.sync.dma_start(wq_h[:, 24:32, :], wqh_d[:, 24:32, :])
                            nc.sync.dma_start(wv_l[:, 16:32, :], wvl_d[:, 16:32, :])
                            nc.sync.dma_start(wk_l[:, 16:32, :], wkl_d[:, 16:32, :])
                            nc.sync.dma_start(wq_l[:, 16:24, :], wql_d[:, 16:24, :])
                            nc.sync.dma_start(wq_l[:, 24:32, :], wql_d[:, 24:32, :])
                            nc.sync.dma_start(sl_l[:], hsl_d[:, 16:32, 0:512])
                            nc.sync.dma_start(cos_sb[:], cos_d[:, 0:512])
                            nc.sync.dma_start(sin_sb[:], sin_d[:, 0:512])
                            nc.sync.dma_start(tri_sb[:], tri_d[:])
                        else:
                            nc.sync.dma_start(
                                sl_h[:], hsh_d[:, ko : ko + 16, c0 : c0 + 512]
                            )
                            nc.sync.dma_start(
                                sl_l[:], hsl_d[:, ko : ko + 16, c0 : c0 + 512]
                            )

                        # targets: (psum, w_hi sel, w_lo sel) in finish order
                        def tsel(w, hd):
                            if hd is None:
                                return lambda k0: w[:, k0 : k0 + 2, :]
                            return lambda k0: w[:, k0 : k0 + 2, hd * 128 : (hd + 1) * 128]

                        targets = [
                            (vtp, tsel(wv_h, None), tsel(wv_l, None)),
                            (kp, tsel(wk_h, None), tsel(wk_l, None)),
                        ] + [(qps[hd], tsel(wq_h, hd), tsel(wq_l, hd)) for hd in range(4)]

                        # main terms (x_hi * w_hi)
                        for ti, (ps, whi, wlo) in enumerate(targets):
                            for p in range(8):
                                nc.tensor.matmul(
                                    ps[:],
                                    whi(ko + 2 * p),
                                    sl_h[:, 2 * p : 2 * p + 2, :],
                                    start=(half == 0 and p == 0),
                                    stop=False,
                                    perf_mode=PM.DoubleRow,
                                )
                        # corr2 (x_hi * w_lo)
                        for ps, whi, wlo in targets:
                            for p in range(8):
                                nc.tensor.matmul(
                                    ps[:],
                                    wlo(ko + 2 * p),
                                    sl_h[:, 2 * p : 2 * p + 2, :],
                                    start=False,
                                    stop=False,
                                    perf_mode=PM.DoubleRow,
                                )
                        # corr1 (x_lo * w_hi)
                        for ps, whi, wlo in targets:
                            for p in range(8):
                                nc.tensor.matmul(
                                    ps[:],
                                    whi(ko + 2 * p),
                                    sl_l[:, 2 * p : 2 * p + 2, :],
                                    start=False,
                                    stop=(half == 1 and p == 7),
                                    perf_mode=PM.DoubleRow,
                                )
                    # free PSUM banks fast: copies to bf16 scratch (aligned
                    # on Act + half-swapped on DVE, all emitted before any
                    # rope so every bank's reads complete early); rope then
                    # reads scratch with partition-aligned DVE ops.
                    def ps_copy(ps, nm, pool=None, pbufs=2):
                        pl = pool if pool is not None else ascr
                        asc = pl.tile([128, 512], dt.bfloat16, name=nm, bufs=pbufs)
                        asw = pl.tile([128, 512], dt.bfloat16, name=nm + "w", bufs=pbufs)
                        nc.scalar.activation(asc[:], ps[:], Act.Copy)
                        # swapped halves from the SBUF copy (not PSUM) so the
                        # PSUM bank frees after the single Act read
                        nc.vector.tensor_copy(asw[0:64, :], asc[64:128, :])
                        nc.vector.tensor_copy(asw[64:128, :], asc[0:64, :])
                        return asc, asw

                    ks = ps_copy(kp, "ksc")
                    vt_sb = ascr.tile([128, 512], dt.bfloat16, bufs=2)
                    nc.scalar.activation(vt_sb[:], vtp[:], Act.Copy)
                    if tb == 7:
                        qs = [
                            ps_copy(qps[_h], f"q7sc{_h}", pool=pp, pbufs=1)
                            for _h in range(4)
                        ]
                    else:
                        qs = [ps_copy(qps[_h], f"qsc{_h}") for _h in range(4)]
                    rope(None, tb, ks[0], ks[1], cos_sb, sin_sb)
                    for s4 in range(4):
                        nc.sync.dma_start_transpose(
                            Vts[tb * 4 + s4][:, 0:128],
                            vt_sb[:, s4 * 128 : (s4 + 1) * 128],
                        )
                    if tb == 7:
                        # defer tb7 q ropes into Phase B (consumers are the
                        # last attention group); keeps DVE clear at the
                        # phase boundary
                        cs7 = pp.tile([128, 512], dt.bfloat16, name="cs7")
                        sn7 = pp.tile([128, 512], dt.bfloat16, name="sn7")
                        nc.vector.tensor_copy(cs7[:], cos_sb[:])
                        nc.vector.tensor_copy(sn7[:], sin_sb[:])
                        defer7.extend(
                            (hd, qs[hd][0], qs[hd][1]) for hd in range(4)
                        )
                    else:
                        for hd in range(4):
                            rope(hd, tb, qs[hd][0], qs[hd][1], cos_sb, sin_sb)

            # ---------------- Phase B (attention) + woven Phase C (o_proj) ---
            with (
                tc.tile_pool(name="cw", bufs=1) as cw,
                tc.tile_pool(name="otp", bufs=6) as otp,
                tc.tile_pool(name="stgp", bufs=6) as stgp,
                tc.tile_pool(name="bpt", bufs=18) as bpt,
                tc.tile_pool(name="brp", bufs=3) as brp,
                tc.tile_pool(name="brc", bufs=8) as brc,
                tc.tile_pool(name="cy", bufs=2) as cy,
                tc.tile_pool(name="drp", bufs=2) as drp,
                tc.tile_pool(name="pss", bufs=2, space="PSUM") as pss,
                tc.tile_pool(name="pog", bufs=2, space="PSUM") as pog,
                tc.tile_pool(name="psy", bufs=2, space="PSUM") as psy,
            ):
                wo_h = cw.tile([128, 4, H], dt.float8e4)
                wo_l = cw.tile([128, 4, H], dt.float8e4)
                for hh in range(4):
                    nc.sync.dma_start(wo_h[:, hh, :], woh_d[:, hh, :])
                for hh in range(4):
                    nc.sync.dma_start(wo_l[:, hh, :], wol_d[:, hh, :])

                cqueue = []
                cstate = {"ysb": None, "units": 0}

                def emit_c_block(force=False):
                    if not cqueue:
                        return
                    i, cb, tag = cqueue[0]
                    if not force and cstate["units"] < tag + 2:
                        return
                    cqueue.pop(0)
                    if cb == 0:
                        cstate["ysb"] = cy.tile([128, H], dt.bfloat16, name="ysb")
                    ysb = cstate["ysb"]
                    yp = psy.tile([128, 512], dt.float32, name="yp")
                    for hp in range(2):
                        nc.tensor.matmul(
                            yp[:],
                            OHts[i][:, 2 * hp : 2 * hp + 2, :],
                            wo_h[:, 2 * hp : 2 * hp + 2, cb * 512 : (cb + 1) * 512],
                            start=(hp == 0),
                            stop=False,
                            perf_mode=PM.DoubleRow,
                        )
                    for hp in range(2):
                        nc.tensor.matmul(
                            yp[:],
                            OLts[i][:, 2 * hp : 2 * hp + 2, :],
                            wo_h[:, 2 * hp : 2 * hp + 2, cb * 512 : (cb + 1) * 512],
                            start=False,
                            stop=False,
                            perf_mode=PM.DoubleRow,
                        )
                    for hp in range(2):
                        nc.tensor.matmul(
                            yp[:],
                            OHts[i][:, 2 * hp : 2 * hp + 2, :],
                            wo_l[:, 2 * hp : 2 * hp + 2, cb * 512 : (cb + 1) * 512],
                            start=False,
                            stop=(hp == 1),
                            perf_mode=PM.DoubleRow,
                        )
                    if force:
                        # post-attention drain: Act engine is idle, and DVE
                        # copy throughput (not PE) bounds the drain rate
                        nc.scalar.activation(
                            ysb[:, cb * 512 : (cb + 1) * 512], yp[:], Act.Copy
                        )
                    else:
                        nc.vector.tensor_copy(
                            ysb[:, cb * 512 : (cb + 1) * 512], yp[:]
                        )
                    w = 2 if i >= 30 else 8
                    if cb % w == w - 1:
                        nc.sync.dma_start(
                            y_d[
                                i * 128 : (i + 1) * 128,
                                (cb - w + 1) * 512 : (cb + 1) * 512,
                            ],
                            ysb[:, (cb - w + 1) * 512 : (cb + 1) * 512],
                        )

                for b in range(2):
                    for g in range(4):
                        for hd in range(4):
                            # [q, d, qc] staging: norms write strided slices,
                            # ONE dma transpose per unit then yields
                            # otg[d, qc, q] (4x fewer HWDGE issues)
                            stg = stgp.tile([128, 4, 128], dt.bfloat16, name="stg")
                            q0 = b * S + g * 512
                            nj = 4 * g + 4
                            diag = list(range(4 * g, nj))
                            off = list(range(0, 4 * g))
                            first_j = off[0] if off else diag[0]
                            last_j = diag[-1]
                            pts = {}
                            ptms = {}

                            def emit_st_exp(j, masked):
                                st = pss.tile([128, 512], dt.float32, name="st")
                                c0 = (j - 4 * g) * 128 if masked else 0
                                nc.tensor.matmul(
                                    st[:, c0:512],
                                    Kts[b * 4 + j // 4][
                                        :, (j % 4) * 128 : (j % 4 + 1) * 128
                                    ],
                                    Qts[hd][b * 4 + g][:, c0:512],
                                    start=True,
                                    stop=True,
                                )
                                pt = bpt.tile([128, 512], dt.bfloat16, name="pt")
                                nc.scalar.activation(
                                    pt[:, c0:512], st[:, c0:512], Act.Exp, scale=SCALE
                                )
                                if masked:
                                    # triangular chunk: keep where qq >= p
                                    # (DVE multiply by precomputed mask; keeps
                                    # Pool out of the attention critical path)
                                    ptm = brp.tile([128, 128], dt.bfloat16, name="ptm")
                                    nc.vector.tensor_mul(
                                        ptm[:], pt[:, c0 : c0 + 128], tri_sb[:]
                                    )
                                    ptms[j] = ptm
                                pts[j] = pt

                            def emit_ot2(j, ogs, p):
                                dg = j - 4 * g
                                for qc in (2 * p, 2 * p + 1):
                                    if dg >= 0 and qc < dg:
                                        continue
                                    if dg >= 0 and qc == dg:
                                        src = ptms[j][:]
                                    else:
                                        src = pts[j][:, qc * 128 : (qc + 1) * 128]
                                    nc.tensor.matmul(
                                        ogs[qc - 2 * p][:, 0:129],
                                        src,
                                        Vts[b * 16 + j][:, 0:129],
                                        start=(j == first_j),
                                        stop=(dg == qc),
                                    )

                            def emit_norms(ogs, p):
                                for qc in (2 * p, 2 * p + 1):
                                    og = ogs[qc - 2 * p]
                                    rc = brc.tile([128, 1], dt.float32, name="rc")
                                    nc.vector.reciprocal(rc[:], og[:, 128:129])
                                    nc.vector.tensor_scalar_mul(
                                        stg[:, qc, :], og[:, 0:128], rc[:]
                                    )

                            ogs_a = [
                                pog.tile([128, 132], dt.float32, name=f"og{_q}")
                                for _q in range(2)
                            ]
                            if not off:
                                for j in diag:
                                    emit_st_exp(j, True)
                                    emit_c_block()
                            for idx, j in enumerate(off):
                                emit_st_exp(j, False)
                                # interleave diag scores so the first off
                                # chunk's exp stays near the Act queue head
                                if idx < 4:
                                    emit_st_exp(diag[idx], True)
                                if idx > 1:
                                    emit_c_block()
                                    emit_ot2(off[idx - 2], ogs_a, 0)
                            if len(off) > 1:
                                emit_c_block()
                                emit_ot2(off[-2], ogs_a, 0)
                            if off:
                                emit_c_block()
                                emit_ot2(off[-1], ogs_a, 0)
                            for j in diag:
                                emit_ot2(j, ogs_a, 0)
                                if not off:
                                    emit_c_block()
                            emit_norms(ogs_a, 0)
                            emit_c_block()
                            ogs_b = [
                                pog.tile([128, 132], dt.float32, name=f"og{_q}")
                                for _q in range(2)
                            ]
                            for j in off:
                                emit_ot2(j, ogs_b, 1)
                            for j in diag:
                                emit_ot2(j, ogs_b, 1)
                            emit_norms(ogs_b, 1)
                            otg = otp.tile([128, 4, 128], dt.bfloat16, name="otg")
                            nc.sync.dma_start_transpose(otg[:], stg[:])
                            for qc in range(4):
                                i = b * 16 + g * 4 + qc
                                # fp8 hi/lo split on the (mostly idle) gpsimd
                                nc.gpsimd.tensor_copy(
                                    OHts[i][:, hd, :], otg[:, qc, :]
                                )
                                nc.gpsimd.tensor_sub(
                                    OLts[i][:, hd, :],
                                    otg[:, qc, :],
                                    OHts[i][:, hd, :],
                                )
                            emit_c_block()
                            emit_c_block()
                            cstate["units"] += 1
                        for ii in range(4):
                            for cb in range(8):
                                cqueue.append(
                                    (b * 16 + g * 4 + ii, cb, cstate["units"])
                                )
                        # one deferred tb7 q-rope per group boundary, on the
                        # mostly idle gpsimd engine
                        if defer7 and cstate["units"] >= 8:
                            dhd, dasc, dasw = defer7.pop(0)
                            dtc = drp.tile([128, 512], dt.bfloat16, name="dtc")
                            dts = drp.tile([128, 512], dt.bfloat16, name="dts")
                            nc.gpsimd.tensor_mul(dtc[:], dasc[:], cs7[:])
                            nc.gpsimd.tensor_mul(dts[:], dasw[:], sn7[:])
                            nc.gpsimd.tensor_sub(
                                Qts[dhd][7][0:64, :], dtc[0:64, :], dts[0:64, :]
                            )
                            nc.gpsimd.tensor_add(
                                Qts[dhd][7][64:128, :], dtc[64:128, :], dts[64:128, :]
                            )
                while cqueue:
                    emit_c_block(force=True)

    nc.compile()
    return nc


def _split8(x, s):
    hi = (x * s).astype(F8)
    lo = ((x * s) - hi.astype(np.float32)).astype(F8)
    return hi, lo


def prep_inputs(inputs):
    hs = np.asarray(inputs["hidden_states"], np.float32)
    cos = np.asarray(inputs["cos"], np.float32)
    sin = np.asarray(inputs["sin"], np.float32)
    wq = np.asarray(inputs["wq"], np.float32)
    wk = np.asarray(inputs["wk"], np.float32)
    wv = np.asarray(inputs["wv"], np.float32)
    wo = np.asarray(inputs["wo"], np.float32)

    hsT = hs.reshape(T, H).T  # [H, T]
    hsT_p = hsT.reshape(KBLK, 128, T).transpose(1, 0, 2)
    hsh, hsl = _split8(hsT_p, XS)
    cosT = (cos.transpose(2, 0, 1).reshape(128, T) * INV_SCALE).astype(BF16)
    sinT = (sin.transpose(2, 0, 1).reshape(128, T) * INV_SCALE).astype(BF16)

    in_maps = []
    for c in range(NCORES):
        wq_c = wq[:, c * 512 : (c + 1) * 512]
        wk_c = wk[:, c * 128 : (c + 1) * 128]
        wv_c = wv[:, c * 128 : (c + 1) * 128]
        wo_c = wo[c * 512 : (c + 1) * 512, :]
        wqh, wql = _split8(wq_c.reshape(KBLK, 128, 512).transpose(1, 0, 2), WS)
        wkh, wkl = _split8(wk_c.reshape(KBLK, 128, 128).transpose(1, 0, 2), WS)
        wvh, wvl = _split8(wv_c.reshape(KBLK, 128, 128).transpose(1, 0, 2), WS)
        woh, wol = _split8(wo_c.reshape(4, 128, H).transpose(1, 0, 2), WS)
        tri = (np.arange(128)[None, :] >= np.arange(128)[:, None]).astype(BF16)
        in_maps.append(
            {
                "tri": tri,
                "hsh": hsh,
                "hsl": hsl,
                "cosT": cosT,
                "sinT": sinT,
                "wqh": wqh,
                "wql": wql,
                "wkh": wkh,
                "wkl": wkl,
                "wvh": wvh,
                "wvl": wvl,
                "woh": woh,
                "wol": wol,
            }
        )
    return in_maps


def kernel(**inputs):
    global _NC
    from concourse.bass_utils import run_bass_kernel_spmd

    if _NC is None:
        _NC = build_nc()
    in_maps = prep_inputs(inputs)
    res = run_bass_kernel_spmd(_NC, in_maps, list(range(NCORES)))
    y = np.zeros((T, H), np.float32)
    for c in range(NCORES):
        y += res.results[c]["y"].astype(np.float32)
    y *= INV_SCALE
    return y.reshape(B, S, H).astype(np.float32)


# revision 37
# speedup vs baseline: 1.0022x; 1.0022x over previous
import sys

if "/opt/trn_rl_repo" not in sys.path:
    sys.path.insert(0, "/opt/trn_rl_repo")

import numpy as np
import ml_dtypes

BF16 = ml_dtypes.bfloat16
F8 = ml_dtypes.float8_e4m3
B, S, H = 2, 2048, 4096
NH, NKV, D = 32, 8, 128
T = B * S
KBLK = H // 128  # 32
SCALE = float(D) ** -0.5
NCORES = 8
XS = 32.0  # activation fp8 scale (2^5)
WS = 512.0  # weight fp8 scale (2^9)
INV_SCALE = 1.0 / (XS * WS)  # 2^-14

_NC = None


def build_nc():
    from concourse import bacc, tile, mybir

    dt = mybir.dt
    Act = mybir.ActivationFunctionType
    Alu = mybir.AluOpType
    PM = mybir.MatmulPerfMode

    nc = bacc.Bacc("TRN2", target_bir_lowering=False, debug=False, num_devices=NCORES)

    hsh_d = nc.dram_tensor("hsh", [128, KBLK, T], dt.float8e4, kind="ExternalInput")
    hsl_d = nc.dram_tensor("hsl", [128, KBLK, T], dt.float8e4, kind="ExternalInput")
    # cos/sin pre-scaled by 2^-14 on host (cancels the fp8 scale in q/k psums)
    cos_d = nc.dram_tensor("cosT", [128, T], dt.bfloat16, kind="ExternalInput")
    sin_d = nc.dram_tensor("sinT", [128, T], dt.bfloat16, kind="ExternalInput")
    wqh_d = nc.dram_tensor("wqh", [128, KBLK, 512], dt.float8e4, kind="ExternalInput")
    wql_d = nc.dram_tensor("wql", [128, KBLK, 512], dt.float8e4, kind="ExternalInput")
    wkh_d = nc.dram_tensor("wkh", [128, KBLK, 128], dt.float8e4, kind="ExternalInput")
    wkl_d = nc.dram_tensor("wkl", [128, KBLK, 128], dt.float8e4, kind="ExternalInput")
    wvh_d = nc.dram_tensor("wvh", [128, KBLK, 128], dt.float8e4, kind="ExternalInput")
    wvl_d = nc.dram_tensor("wvl", [128, KBLK, 128], dt.float8e4, kind="ExternalInput")
    woh_d = nc.dram_tensor("woh", [128, 4, H], dt.float8e4, kind="ExternalInput")
    wol_d = nc.dram_tensor("wol", [128, 4, H], dt.float8e4, kind="ExternalInput")
    tri_d = nc.dram_tensor("tri", [128, 128], dt.bfloat16, kind="ExternalInput")
    # y carries a 2^14 scale; host divides after the cross-core reduction
    y_d = nc.dram_tensor("y", [T, H], dt.bfloat16, kind="ExternalOutput")

    with tile.TileContext(nc) as tc:
        with tc.tile_pool(name="persist", bufs=1) as pp:
            # per-chunk tiles => precise (non-coarse) dependency tracking
            Qts = [
                [pp.tile([128, 512], dt.bfloat16, name=f"q{h}_{t}") for t in range(8)]
                for h in range(4)
            ]
            Kts = [pp.tile([128, 512], dt.bfloat16, name=f"k{t}") for t in range(8)]
            # V augmented with a 512-valued column at free idx 128: the
            # denominator then carries 512*sum(p) while numerator carries
            # 2^14*sum(p*v), so the normalized output lands pre-scaled by 2^5
            # (the fp8 scale wanted for the o_proj split) for free.
            Vts = [pp.tile([128, 132], dt.bfloat16, name=f"v{c}") for c in range(32)]
            OHts = [pp.tile([128, 4, 128], dt.float8e4, name=f"oh{i}") for i in range(32)]
            OLts = [pp.tile([128, 4, 128], dt.float8e4, name=f"ol{i}") for i in range(32)]
            tri_sb = pp.tile([128, 128], dt.bfloat16)
            for c in range(32):
                nc.vector.memset(Vts[c][:, 128:132], XS * WS / XS)  # 512.0

            # ---------------- Phase A: QKV projections + RoPE ----------------
            with (
                tc.tile_pool(name="aw", bufs=1) as aw,
                tc.tile_pool(name="slabp", bufs=2) as slabp,
                tc.tile_pool(name="ascr", bufs=3) as ascr,
                tc.tile_pool(name="psa", bufs=1, space="PSUM") as psa,
            ):
                wq_h = aw.tile([128, KBLK, 512], dt.float8e4)
                wq_l = aw.tile([128, KBLK, 512], dt.float8e4)
                wk_h = aw.tile([128, KBLK, 128], dt.float8e4)
                wk_l = aw.tile([128, KBLK, 128], dt.float8e4)
                wv_h = aw.tile([128, KBLK, 128], dt.float8e4)
                wv_l = aw.tile([128, KBLK, 128], dt.float8e4)

                def rope(hd, tb, asc, asw, cs, sn):
                    # dst[:64] = x[:64]*cos[:64] - x[64:]*sin[:64]
                    # dst[64:] = x[64:]*cos[64:] + x[:64]*sin[64:]
                    # asw holds x with the halves pre-swapped, so both muls
                    # and the sub/add are partition-aligned on DVE.
                    tmpc = ascr.tile([128, 512], dt.bfloat16, bufs=2)
                    tmps = ascr.tile([128, 512], dt.bfloat16, bufs=2)
                    nc.vector.tensor_mul(tmpc[:], asc[:], cs[:])
                    nc.vector.tensor_mul(tmps[:], asw[:], sn[:])
                    if hd is None:
                        d_lo = Kts[tb][0:64, :]
                        d_hi = Kts[tb][64:128, :]
                    else:
                        d_lo = Qts[hd][tb][0:64, :]
                        d_hi = Qts[hd][tb][64:128, :]
                    nc.vector.tensor_sub(d_lo, tmpc[0:64, :], tmps[0:64, :])
                    nc.vector.tensor_add(d_hi, tmpc[64:128, :], tmps[64:128, :])

                for tb in range(8):
                    c0 = tb * 512
                    kp = psa.tile([128, 512], dt.float32)
                    vtp = psa.tile([128, 512], dt.float32)
                    qps = [
                        psa.tile([128, 512], dt.float32, name=f"qp{_h}")
                        for _h in range(4)
                    ]
                    cos_sb = ascr.tile([128, 512], dt.bfloat16, name="cos_sb", bufs=2)
                    sin_sb = ascr.tile([128, 512], dt.bfloat16, name="sin_sb", bufs=2)
                    if tb > 0:
                        nc.sync.dma_start(cos_sb[:], cos_d[:, c0 : c0 + 512])
                        nc.sync.dma_start(sin_sb[:], sin_d[:, c0 : c0 + 512])
                    for half in range(2):
                        ko = half * 16
                        sl_h = slabp.tile([128, 16, 512], dt.float8e4, name="slh")
                        sl_l = slabp.tile([128, 16, 512], dt.float8e4, name="sll")
                        if tb == 0 and half == 0:
                            nc.sync.dma_start(sl_h[:, 0:2, :], hsh_d[:, 0:2, 0:512])
                            nc.sync.dma_start(wv_h[:, 0:4, :], wvh_d[:, 0:4, :])
                            nc.sync.dma_start(sl_h[:, 2:8, :], hsh_d[:, 2:8, 0:512])
                            nc.sync.dma_start(wv_h[:, 4:16, :], wvh_d[:, 4:16, :])
                            nc.sync.dma_start(sl_h[:, 8:16, :], hsh_d[:, 8:16, 0:512])
                            nc.sync.dma_start(wk_h[:, 0:16, :], wkh_d[:, 0:16, :])
                            nc.sync.dma_start(wq_h[:, 0:8, :], wqh_d[:, 0:8, :])
                            nc.sync.dma_start(wq_h[:, 8:16, :], wqh_d[:, 8:16, :])
                            nc.sync.dma_start(wv_l[:, 0:16, :], wvl_d[:, 0:16, :])
                            nc.sync.dma_start(wk_l[:, 0:16, :], wkl_d[:, 0:16, :])
                            nc.sync.dma_start(wq_l[:, 0:8, :], wql_d[:, 0:8, :])
                            nc.sync.dma_start(wq_l[:, 8:16, :], wql_d[:, 8:16, :])
                            nc.sync.dma_start(sl_l[:], hsl_d[:, 0:16, 0:512])
                        elif tb == 0 and half == 1:
                            nc.sync.dma_start(sl_h[:], hsh_d[:, 16:32, 0:512])
                            nc.sync.dma_start(wv_h[:, 16:32, :], wvh_d[:, 16:32, :])
                            nc.sync.dma_start(wk_h[:, 16:32, :], wkh_d[:, 16:32, :])
                            nc.sync.dma_start(wq_h[:, 16:24, :], wqh_d[:, 16:24, :])
                            nc.sync.dma_start(wq_h[:, 24:32, :], wqh_d[:, 24:32, :])
                            nc.sync.dma_start(wv_l[:, 16:32, :], wvl_d[:, 16:32, :])
                            nc.sync.dma_start(wk_l[:, 16:32, :], wkl_d[:, 16:32, :])
                            nc.sync.dma_start(wq_l[:, 16:24, :], wql_d[:, 16:24, :])
                            nc.sync.dma_start(wq_l[:, 24:32, :], wql_d[:, 24:32, :])
                            nc.sync.dma_start(sl_l[:], hsl_d[:, 16:32, 0:512])
                            nc.sync.dma_start(cos_sb[:], cos_d[:, 0:512])
                            nc.sync.dma_start(sin_sb[:], sin_d[:, 0:512])
                            nc.sync.dma_start(tri_sb[:], tri_d[:])
                        else:
                            nc.sync.dma_start(
                                sl_h[:], hsh_d[:, ko : ko + 16, c0 : c0 + 512]
                            )
                            nc.sync.dma_start(
                                sl_l[:], hsl_d[:, ko : ko + 16, c0 : c0 + 512]
                            )

                        # targets: (psum, w_hi sel, w_lo sel) in finish order
                        def tsel(w, hd):
                            if hd is None:
                                return lambda k0: w[:, k0 : k0 + 2, :]
                            return lambda k0: w[:, k0 : k0 + 2, hd * 128 : (hd + 1) * 128]

                        targets = [
                            (vtp, tsel(wv_h, None), tsel(wv_l, None)),
                            (kp, tsel(wk_h, None), tsel(wk_l, None)),
                        ] + [(qps[hd], tsel(wq_h, hd), tsel(wq_l, hd)) for hd in range(4)]

                        # main terms (x_hi * w_hi)
                        for ti, (ps, whi, wlo) in enumerate(targets):
                            for p in range(8):
                                nc.tensor.matmul(
                                    ps[:],
                                    whi(ko + 2 * p),
                                    sl_h[:, 2 * p : 2 * p + 2, :],
                                    start=(half == 0 and p == 0),
                                    stop=False,
                                    perf_mode=PM.DoubleRow,
                                )
                        # corr2 (x_hi * w_lo)
                        for ps, whi, wlo in targets:
                            for p in range(8):
                                nc.tensor.matmul(
                                    ps[:],
                                    wlo(ko + 2 * p),
                                    sl_h[:, 2 * p : 2 * p + 2, :],
                                    start=False,
                                    stop=False,
                                    perf_mode=PM.DoubleRow,
                                )
                        # corr1 (x_lo * w_hi)
                        for ps, whi, wlo in targets:
                            for p in range(8):
                                nc.tensor.matmul(
                                    ps[:],
                                    whi(ko + 2 * p),
                                    sl_l[:, 2 * p : 2 * p + 2, :],
                                    start=False,
                                    stop=(half == 1 and p == 7),
                                    perf_mode=PM.DoubleRow,
                                )
                    # free PSUM banks fast: copies to bf16 scratch (aligned
                    # on Act + half-swapped on DVE, all emitted before any
                    # rope so every bank's reads complete early); rope then
                    # reads scratch with partition-aligned DVE ops.
                    def ps_copy(ps, nm):
                        asc = ascr.tile([128, 512], dt.bfloat16, name=nm, bufs=2)
                        asw = ascr.tile([128, 512], dt.bfloat16, name=nm + "w", bufs=2)
                        nc.scalar.activation(asc[:], ps[:], Act.Copy)
                        # swapped halves from the SBUF copy (not PSUM) so the
                        # PSUM bank frees after the single Act read
                        nc.vector.tensor_copy(asw[0:64, :], asc[64:128, :])
                        nc.vector.tensor_copy(asw[64:128, :], asc[0:64, :])
                        return asc, asw

                    ks = ps_copy(kp, "ksc")
                    vt_sb = ascr.tile([128, 512], dt.bfloat16, bufs=2)
                    nc.scalar.activation(vt_sb[:], vtp[:], Act.Copy)
                    qs = [ps_copy(qps[_h], f"qsc{_h}") for _h in range(4)]
                    rope(None, tb, ks[0], ks[1], cos_sb, sin_sb)
                    for s4 in range(4):
                        nc.sync.dma_start_transpose(
                            Vts[tb * 4 + s4][:, 0:128],
                            vt_sb[:, s4 * 128 : (s4 + 1) * 128],
                        )
                    for hd in range(4):
                        rope(hd, tb, qs[hd][0], qs[hd][1], cos_sb, sin_sb)

            # ---------------- Phase B (attention) + woven Phase C (o_proj) ---
            with (
                tc.tile_pool(name="cw", bufs=1) as cw,
                tc.tile_pool(name="otp", bufs=6) as otp,
                tc.tile_pool(name="stgp", bufs=6) as stgp,
                tc.tile_pool(name="bpt", bufs=18) as bpt,
                tc.tile_pool(name="brp", bufs=3) as brp,
                tc.tile_pool(name="brc", bufs=8) as brc,
                tc.tile_pool(name="cy", bufs=2) as cy,
                tc.tile_pool(name="pss", bufs=2, space="PSUM") as pss,
                tc.tile_pool(name="pog", bufs=2, space="PSUM") as pog,
                tc.tile_pool(name="psy", bufs=2, space="PSUM") as psy,
            ):
                wo_h = cw.tile([128, 4, H], dt.float8e4)
                wo_l = cw.tile([128, 4, H], dt.float8e4)
                for hh in range(4):
                    nc.sync.dma_start(wo_h[:, hh, :], woh_d[:, hh, :])
                for hh in range(4):
                    nc.sync.dma_start(wo_l[:, hh, :], wol_d[:, hh, :])

                cqueue = []
                cstate = {"ysb": None, "units": 0}

                def emit_c_block(force=False):
                    if not cqueue:
                        return
                    i, cb, tag = cqueue[0]
                    if not force and cstate["units"] < tag + 2:
                        return
                    cqueue.pop(0)
                    if cb == 0:
                        cstate["ysb"] = cy.tile([128, H], dt.bfloat16, name="ysb")
                    ysb = cstate["ysb"]
                    yp = psy.tile([128, 512], dt.float32, name="yp")
                    for hp in range(2):
                        nc.tensor.matmul(
                            yp[:],
                            OHts[i][:, 2 * hp : 2 * hp + 2, :],
                            wo_h[:, 2 * hp : 2 * hp + 2, cb * 512 : (cb + 1) * 512],
                            start=(hp == 0),
                            stop=False,
                            perf_mode=PM.DoubleRow,
                        )
                    for hp in range(2):
                        nc.tensor.matmul(
                            yp[:],
                            OLts[i][:, 2 * hp : 2 * hp + 2, :],
                            wo_h[:, 2 * hp : 2 * hp + 2, cb * 512 : (cb + 1) * 512],
                            start=False,
                            stop=False,
                            perf_mode=PM.DoubleRow,
                        )
                    for hp in range(2):
                        nc.tensor.matmul(
                            yp[:],
                            OHts[i][:, 2 * hp : 2 * hp + 2, :],
                            wo_l[:, 2 * hp : 2 * hp + 2, cb * 512 : (cb + 1) * 512],
                            start=False,
                            stop=(hp == 1),
                            perf_mode=PM.DoubleRow,
                        )
                    if force:
                        # post-attention drain: Act engine is idle, and DVE
                        # copy throughput (not PE) bounds the drain rate
                        nc.scalar.activation(
                            ysb[:, cb * 512 : (cb + 1) * 512], yp[:], Act.Copy
                        )
                    else:
                        nc.vector.tensor_copy(
                            ysb[:, cb * 512 : (cb + 1) * 512], yp[:]
                        )
                    w = 2 if i >= 30 else 8
                    if cb % w == w - 1:
                        nc.sync.dma_start(
                            y_d[
                                i * 128 : (i + 1) * 128,
                                (cb - w + 1) * 512 : (cb + 1) * 512,
                            ],
                            ysb[:, (cb - w + 1) * 512 : (cb + 1) * 512],
                        )

                for b in range(2):
                    for g in range(4):
                        for hd in range(4):
                            # [q, d, qc] staging: norms write strided slices,
                            # ONE dma transpose per unit then yields
                            # otg[d, qc, q] (4x fewer HWDGE issues)
                            stg = stgp.tile([128, 4, 128], dt.bfloat16, name="stg")
                            q0 = b * S + g * 512
                            nj = 4 * g + 4
                            diag = list(range(4 * g, nj))
                            off = list(range(0, 4 * g))
                            first_j = off[0] if off else diag[0]
                            last_j = diag[-1]
                            pts = {}
                            ptms = {}

                            def emit_st_exp(j, masked):
                                st = pss.tile([128, 512], dt.float32, name="st")
                                c0 = (j - 4 * g) * 128 if masked else 0
                                nc.tensor.matmul(
                                    st[:, c0:512],
                                    Kts[b * 4 + j // 4][
                                        :, (j % 4) * 128 : (j % 4 + 1) * 128
                                    ],
                                    Qts[hd][b * 4 + g][:, c0:512],
                                    start=True,
                                    stop=True,
                                )
                                pt = bpt.tile([128, 512], dt.bfloat16, name="pt")
                                nc.scalar.activation(
                                    pt[:, c0:512], st[:, c0:512], Act.Exp, scale=SCALE
                                )
                                if masked:
                                    # triangular chunk: keep where qq >= p
                                    # (DVE multiply by precomputed mask; keeps
                                    # Pool out of the attention critical path)
                                    ptm = brp.tile([128, 128], dt.bfloat16, name="ptm")
                                    nc.vector.tensor_mul(
                                        ptm[:], pt[:, c0 : c0 + 128], tri_sb[:]
                                    )
                                    ptms[j] = ptm
                                pts[j] = pt

                            def emit_ot2(j, ogs, p):
                                dg = j - 4 * g
                                for qc in (2 * p, 2 * p + 1):
                                    if dg >= 0 and qc < dg:
                                        continue
                                    if dg >= 0 and qc == dg:
                                        src = ptms[j][:]
                                    else:
                                        src = pts[j][:, qc * 128 : (qc + 1) * 128]
                                    nc.tensor.matmul(
                                        ogs[qc - 2 * p][:, 0:129],
                                        src,
                                        Vts[b * 16 + j][:, 0:129],
                                        start=(j == first_j),
                                        stop=(dg == qc),
                                    )

                            def emit_norms(ogs, p):
                                for qc in (2 * p, 2 * p + 1):
                                    og = ogs[qc - 2 * p]
                                    rc = brc.tile([128, 1], dt.float32, name="rc")
                                    nc.vector.reciprocal(rc[:], og[:, 128:129])
                                    nc.vector.tensor_scalar_mul(
                                        stg[:, qc, :], og[:, 0:128], rc[:]
                                    )

                            ogs_a = [
                                pog.tile([128, 132], dt.float32, name=f"og{_q}")
                                for _q in range(2)
                            ]
                            if not off:
                                for j in diag:
                                    emit_st_exp(j, True)
                                    emit_c_block()
                            for idx, j in enumerate(off):
                                emit_st_exp(j, False)
                                # interleave diag scores so the first off
                                # chunk's exp stays near the Act queue head
                                if idx < 4:
                                    emit_st_exp(diag[idx], True)
                                if idx > 1:
                                    emit_c_block()
                                    emit_ot2(off[idx - 2], ogs_a, 0)
                            if len(off) > 1:
                                emit_c_block()
                                emit_ot2(off[-2], ogs_a, 0)
                            if off:
                                emit_c_block()
                                emit_ot2(off[-1], ogs_a, 0)
                            for j in diag:
                                emit_ot2(j, ogs_a, 0)
                                if not off:
                                    emit_c_block()
                            emit_norms(ogs_a, 0)
                            emit_c_block()
                            ogs_b = [
                                pog.tile([128, 132], dt.float32, name=f"og{_q}")
                                for _q in range(2)
                            ]
                            for j in off:
                                emit_ot2(j, ogs_b, 1)
                            for j in diag:
                                emit_ot2(j, ogs_b, 1)
                            emit_norms(ogs_b, 1)
                            otg = otp.tile([128, 4, 128], dt.bfloat16, name="otg")
                            nc.sync.dma_start_transpose(otg[:], stg[:])
                            for qc in range(4):
                                i = b * 16 + g * 4 + qc
                                # fp8 hi/lo split on the (mostly idle) gpsimd
                                nc.gpsimd.tensor_copy(
                                    OHts[i][:, hd, :], otg[:, qc, :]
                                )
                                nc.gpsimd.tensor_sub(
                                    OLts[i][:, hd, :],
                                    otg[:, qc, :],
                                    OHts[i][:, hd, :],
                                )
                            emit_c_block()
                            emit_c_block()
                            cstate["units"] += 1
                        for ii in range(4):
                            for cb in range(8):
                                cqueue.append(
                                    (b * 16 + g * 4 + ii, cb, cstate["units"])
                                )
                while cqueue:
                    emit_c_block(force=True)

    nc.compile()
    return nc


def _split8(x, s):
    hi = (x * s).astype(F8)
    lo = ((x * s) - hi.astype(np.float32)).astype(F8)
    return hi, lo


def prep_inputs(inputs):
    hs = np.asarray(inputs["hidden_states"], np.float32)
    cos = np.asarray(inputs["cos"], np.float32)
    sin = np.asarray(inputs["sin"], np.float32)
    wq = np.asarray(inputs["wq"], np.float32)
    wk = np.asarray(inputs["wk"], np.float32)
    wv = np.asarray(inputs["wv"], np.float32)
    wo = np.asarray(inputs["wo"], np.float32)

    hsT = hs.reshape(T, H).T  # [H, T]
    hsT_p = hsT.reshape(KBLK, 128, T).transpose(1, 0, 2)
    hsh, hsl = _split8(hsT_p, XS)
    cosT = (cos.transpose(2, 0, 1).reshape(128, T) * INV_SCALE).astype(BF16)
    sinT = (sin.transpose(2, 0, 1).reshape(128, T) * INV_SCALE).astype(BF16)

    in_maps = []
    for c in range(NCORES):
        wq_c = wq[:, c * 512 : (c + 1) * 512]
        wk_c = wk[:, c * 128 : (c + 1) * 128]
        wv_c = wv[:, c * 128 : (c + 1) * 128]
        wo_c = wo[c * 512 : (c + 1) * 512, :]
        wqh, wql = _split8(wq_c.reshape(KBLK, 128, 512).transpose(1, 0, 2), WS)
        wkh, wkl = _split8(wk_c.reshape(KBLK, 128, 128).transpose(1, 0, 2), WS)
        wvh, wvl = _split8(wv_c.reshape(KBLK, 128, 128).transpose(1, 0, 2), WS)
        woh, wol = _split8(wo_c.reshape(4, 128, H).transpose(1, 0, 2), WS)
        tri = (np.arange(128)[None, :] >= np.arange(128)[:, None]).astype(BF16)
        in_maps.append(
            {
                "tri": tri,
                "hsh": hsh,
                "hsl": hsl,
                "cosT": cosT,
                "sinT": sinT,
                "wqh": wqh,
                "wql": wql,
                "wkh": wkh,
                "wkl": wkl,
                "wvh": wvh,
                "wvl": wvl,
                "woh": woh,
                "wol": wol,
            }
        )
    return in_maps


def kernel(**inputs):
    global _NC
    from concourse.bass_utils import run_bass_kernel_spmd

    if _NC is None:
        _NC = build_nc()
    in_maps = prep_inputs(inputs)
    res = run_bass_kernel_spmd(_NC, in_maps, list(range(NCORES)))
    y = np.zeros((T, H), np.float32)
    for c in range(NCORES):
        y += res.results[c]["y"].astype(np.float32)
    y *= INV_SCALE
    return y.reshape(B, S, H).astype(np.float32)
